# revision 1
# baseline (speedup 1.0000x reference)
"""De-emphasis IIR x[n] = 0.95*x[n-1] + e[n] over (64, 480000) fp32 — two-engine pipeline (DVE scan + PE block-scan).

Per core (8 rows), two streams, fp16 inputs on device:

  Stream A (Vector engine): row-major overlap-save scan.  Each row
  contributes 16 intervals of M_A cols; 8x16 = 128 partitions; W warm-up
  elements per interval; tensor_tensor_scan in fp16.

  Stream B (Tensor engine): block-scan as two matmuls.  Each row's
  remaining 16*M_B samples are laid out position-major: [128, c] tiles
  where partition i holds sample i of each 128-block, c = 16*M_B/128
  cols per row.  Then
      X[:, m] = T @ E[:, m] + W2 @ E[:, m-1]
  with T[i,j] = 0.95^(i-j) (i>=j), W2[i,j] = 0.95^(128+i-j) — the exact
  cross-block carry to one column back (two columns back is < 1.4e-3 and
  dropped).  A zero column precedes each row's block so row boundaries
  don't couple.  Output lands in PSUM (fp32) and is DMA'd out directly.

  The three DMA queues (sync=SP, scalar=Act, gpsimd=Pool) carry
  loads/stores balanced so every queue finishes by the DVE end.
"""

import numpy as np

COEFF = 0.95
ROWS = 64
N = 480000
N_CORES = 8
RPC = ROWS // N_CORES
NSEG = 16
W = 64

CFG = dict(
    m_a=16080,           # stream-A output cols per partition
    nA=10, firstA=320, lastA=300,
    nBl=8,               # B load chunks
    n_act_bloads=3,      # B loads 0..n-1 on Act, rest on Pool
    sa_act=5,
    sa_queues="sp,sp,sp,sp,gp,gp,sp,act,act,sp",
    taperA=(1300, 700, 300),
    sb_queues=None,
    fp8_from=4,   # stB groups >= this stored as fp8 via gpsimd casting DMA
    b8s_chunk=None, # PE chunk index where fp8 INPUT begins (None = off)
    firstB2=None,   # explicit sizes for the first B loads (early-evac tuning)
    split_stA=None, # (chunk_idx, q_first_half, q_second_half) store split
    la_act=(),      # A-load chunk indices carried by Act's early idle gap
    psum_chunk=512,
    ev_group=4,          # mm chunks per evac/store group (<=4: 8 banks/2)
)

R_DMA2 = 2 * 0.3855
R_DMA4 = 4 * 0.3855
DMA_LAT = 1717.0
DVE_R = 1.0417
PE_R = 0.8333  # 2 matmuls x 0.4167 (full p-state)


def _chunks(total, n, first, last):
    mid = n - 2
    body = (total - first - last) // mid
    sizes = [first] + [body] * mid + [last]
    sizes[-2] += total - sum(sizes)
    return sizes


def _derive(cfg):
    m_a = cfg["m_a"]
    m_b = N // NSEG - m_a          # per-row B cols = 16*m_b
    c = (NSEG * m_b) // 128        # data cols per row (position-major)
    assert (NSEG * m_b) % 128 == 0, (m_a, m_b)
    L_a = m_a + W
    # per row: [zero col, 2 warm-up cols (256 samples before S), c data cols]
    L_pe = RPC * (c + 3)
    return m_a, m_b, c, L_a, L_pe


def _plan(cfg):
    m_a, m_b, c, L_a, L_pe = _derive(cfg)
    taper = cfg.get("taperA")
    if taper:
        body_n = cfg["nA"] - 1 - len(taper)
        rem = L_a - cfg["firstA"] - sum(taper)
        body = rem // body_n
        sizes_a = [cfg["firstA"]] + [body] * body_n + list(taper)
        sizes_a[body_n] += L_a - sum(sizes_a)
    else:
        sizes_a = _chunks(L_a, cfg["nA"], cfg["firstA"], cfg["lastA"])
    offs_a = np.concatenate([[0], np.cumsum(sizes_a)]).astype(int)

    # PE chunks over cols [1, L_pe), each <= psum_chunk
    pc = cfg["psum_chunk"]
    pe_edges = list(range(1, L_pe, pc)) + [L_pe]
    pe_chunks = [(pe_edges[i], pe_edges[i + 1]) for i in range(len(pe_edges) - 1)]

    # fp8-input region starts at a PE chunk edge (b8s = col index)
    b8c = cfg.get("b8s_chunk")
    b8s = pe_edges[b8c] if b8c is not None and b8c < len(pe_chunks) else L_pe

    # B load chunks: fp16 region [0, b8s), fp8 region [b8s-1, L_pe)
    fb = cfg.get("firstB2")
    if fb:
        rest = _chunks(b8s - sum(fb), cfg["nBl"] - len(fb), 648, 648)
        sizes_bl = list(fb) + rest
    else:
        sizes_bl = _chunks(b8s, cfg["nBl"], 648, 648)
    offs_bl = np.concatenate([[0], np.cumsum(sizes_bl)]).astype(int)
    n8l = max(1, (L_pe - b8s) // 2600) if b8s < L_pe else 0
    if n8l:
        f8len = (L_pe - (b8s - 1)) // n8l
        offs_b8 = [b8s - 1 + f8len * i for i in range(n8l)] + [L_pe]

    ops = {}

    def add(name, queue, cost, deps, **meta):
        ops[name] = dict(q=queue, cost=cost, deps=deps, **meta)

    la_act = cfg.get("la_act", ())
    for k, f in enumerate(sizes_a):
        # an Act-carried A-load must queue AFTER Act's urgent early B-loads
        deps = ([f"lB{cfg['n_act_bloads'] - 1}"] if k in la_act else [])
        add(f"lA{k}", "act" if k in la_act else "sp",
            max(f * R_DMA2, 500), deps, dma=True,
            a0=int(offs_a[k]), a1=int(offs_a[k + 1]), kind="loadA")
    for k, f in enumerate(sizes_bl):
        q = "act" if k < cfg["n_act_bloads"] else "gp"
        add(f"lB{k}", q, max(f * R_DMA2, 500), [], dma=True,
            a0=int(offs_bl[k]), a1=int(offs_bl[k + 1]), kind="loadB")
    for k in range(n8l):
        a0, a1 = offs_b8[k], offs_b8[k + 1]
        add(f"l8B{k}", "gp", max((a1 - a0) * 0.3855, 500), [], dma=True,
            a0=a0, a1=a1, kind="loadB8")

    for k, f in enumerate(sizes_a):
        deps = [f"lA{k}"] + ([f"sA{k-1}"] if k else [])
        add(f"sA{k}", "dve", f * DVE_R + 60, deps, dma=False,
            a0=int(offs_a[k]), a1=int(offs_a[k + 1]), kind="scanA", idx=k)
        if k == 0:
            continue  # chunk 0's tiny store is merged into stA1
        s0 = W if k == 1 else int(offs_a[k])
        if int(offs_a[k + 1]) > s0:
            qs = cfg.get("sa_queues", "sp,sp,sp,sp,gp,gp,gp,act,act").split(",")
            q = qs[k % len(qs)]
            sk = cfg.get("split_stA")
            if sk and k == sk[0]:
                mid = (s0 + int(offs_a[k + 1])) // 2
                add(f"stA{k}", sk[1], max((mid - s0) * R_DMA2, 500),
                    [f"sA{k}"], dma=True, a0=s0, a1=mid, kind="storeA")
                add(f"stA{k}b", sk[2], max((int(offs_a[k + 1]) - mid) * R_DMA2,
                    500), [f"sA{k}"], dma=True, a0=mid,
                    a1=int(offs_a[k + 1]), kind="storeA")
            else:
                add(f"stA{k}", q, max((int(offs_a[k + 1]) - s0) * R_DMA2, 500),
                    [f"sA{k}"], dma=True, a0=s0, a1=int(offs_a[k + 1]),
                    kind="storeA")

    # PE chunks; grouped evac on Act; fp16 stores mostly Pool
    nst = len(pe_chunks)
    for k, (a0, a1) in enumerate(pe_chunks):
        if a0 >= b8s:
            deps = [f"l8B{j}" for j in range(n8l)
                    if offs_b8[j] < a1 and offs_b8[j + 1] > a0 - 1]
        else:
            deps = [f"lB{j}" for j in range(len(sizes_bl))
                    if offs_bl[j] < a1 and offs_bl[j + 1] > a0 - 1]
        if k:
            deps.append(f"mm{k-1}")
        # PE p-state: ~mid clock (2x cost) for the first ~3us of PE busy time
        pe_busy_est = sum(ops[f"mm{j}"]["cost"] for j in range(k))
        rate = PE_R * 2.0 if pe_busy_est < 3000.0 else PE_R
        add(f"mm{k}", "pe", (a1 - a0) * rate, deps, dma=False,
            a0=a0, a1=a1, kind="mm", idx=k, grp=0)
    # uniform groups of ev_group chunks (tail variants modeled worse)
    gsizes = []
    rem = nst
    while rem > 0:
        take = min(cfg["ev_group"], rem)
        gsizes.append(take)
        rem -= take
    g_of = []
    for g, sz in enumerate(gsizes):
        g_of += [g] * sz
    for k in range(nst):
        ops[f"mm{k}"]["grp"] = g_of[k]
    ngrp = len(gsizes)
    for g in range(ngrp):
        ks = [k for k in range(nst) if g_of[k] == g]
        a0 = pe_chunks[ks[0]][0]
        a1 = pe_chunks[ks[-1]][1]
        # first evac also pays ACT_TABLE_LOAD (1283 ns) in CoreSim
        tbl = 1283.0 if g == 0 else 0.0
        add(f"ev{g}", "act", (a1 - a0) * 0.8333 + 245 + tbl,
            [f"mm{k}" for k in ks], dma=False, a0=a0, a1=a1, kind="evac",
            idx=g)
        fp8 = g >= cfg.get("fp8_from", 99)
        if fp8:
            q, rate = "gp", 0.3855  # gpsimd casting DMA, 1 B/col
        else:
            sbq = cfg.get("sb_queues")
            q = (sbq.split(",")[g % len(sbq.split(","))] if sbq
                 else ("gp" if g < ngrp - 1 else "act"))
            rate = R_DMA2
        add(f"stB{g}", q, max((a1 - a0) * rate, 500), [f"ev{g}"],
            dma=True, a0=a0, a1=a1, kind="storeB", idx=g, fp8=fp8)

    # greedy queue timeline (sp/act/poolq(gp)/dve/pe); stores with q="any"
    # are placed on the queue giving the earliest start
    qof = {"sp": "sp", "act": "act", "gp": "poolq", "dve": "dve", "pe": "pe"}
    qfree = {v: 0.0 for v in qof.values()}
    done, started, order = {}, {}, []
    remaining = dict(ops)
    DMAQ = ("sp", "act", "poolq")
    while remaining:
        best = None
        for name, op in remaining.items():
            if any(d not in done for d in op["deps"]):
                continue
            dep_ready = max([0.0] + [done[d] for d in op["deps"]])
            if op["q"] == "any":
                qn = min(DMAQ, key=lambda q: max(qfree[q], dep_ready))
            else:
                qn = qof[op["q"]]
            ready = max(qfree[qn], dep_ready)
            key = (ready, 0 if op["kind"].startswith("load") else 1)
            if best is None or key < best[0]:
                best = (key, name, qn, ready)
        _, name, qn, start = best
        op = remaining.pop(name)
        op["q"] = {"poolq": "gp"}.get(qn, qn) if op["q"] == "any" else op["q"]
        end = start + op["cost"]
        qfree[qn] = end
        started[name] = start
        done[name] = end + (DMA_LAT if op["dma"] else 100.0)
        order.append(name)
    makespan = max(done.values()) + 400.0
    return sizes_a, offs_a, sizes_bl, offs_bl, pe_chunks, ops, order, started, makespan, b8s


def plan_makespan(cfg=None):
    return _plan(cfg or CFG)[8]


def _fp8_start(cfg):
    """First output col stored as fp8 (None if fp8 disabled)."""
    ops = _plan(cfg)[5]
    cols = [ops[n]["a0"] for n in ops if n.startswith("stB")
            and ops[n].get("fp8")]
    return min(cols) if cols else None


def _b8_start(cfg):
    return _plan(cfg)[9]


def _weights():
    """lhsT matrices [K=j, M=i] fp16: Wmain[j,i] = 0.95^(i-j) for i>=j,
    W2[j,i] = 0.95^(128+i-j)."""
    i = np.arange(128)[None, :]
    j = np.arange(128)[:, None]
    wmain = np.where(i >= j, COEFF ** (i - j), 0.0)
    w2 = COEFF ** (128 + i - j)
    return wmain.astype(np.float16), w2.astype(np.float16)


_cached = {}


def _build_bass(split_waits=True, cfg=None):
    import concourse.bass as bass
    import concourse.mybir as mybir
    from concourse.tile import TileContext

    cfg = cfg or CFG
    m_a, m_b, c, L_a, L_pe = _derive(cfg)
    (sizes_a, offs_a, sizes_bl, offs_bl, pe_chunks, ops, order, started,
     mk, b8s) = _plan(cfg)

    f16 = mybir.dt.float16
    f32 = mybir.dt.float32
    nc = bass.Bass(trn_type="TRN2")
    xa = nc.dram_tensor("xa", [128, L_a], f16, kind="ExternalInput")
    xb = nc.dram_tensor("xb", [128, max(b8s, 2)], f16, kind="ExternalInput")
    n8 = L_pe - (b8s - 1)
    xb8 = (nc.dram_tensor("xb8", [128, n8], mybir.dt.float8e4,
                          kind="ExternalInput") if b8s < L_pe else None)
    wt = nc.dram_tensor("wt", [128, 256], f16, kind="ExternalInput")
    f8 = mybir.dt.float8e4
    ya = nc.dram_tensor("ya", [128, m_a], f16, kind="ExternalOutput")
    yb = nc.dram_tensor("yb", [128, L_pe], f16, kind="ExternalOutput")
    yb8 = nc.dram_tensor("yb8", [128, L_pe], f8, kind="ExternalOutput")

    with TileContext(nc) as tc:
        with (
            tc.tile_pool(name="coef", bufs=1) as coefp,
            tc.tile_pool(name="bufa", bufs=1) as pa,
            tc.tile_pool(name="bufb", bufs=1) as pb,
            tc.psum_pool(name="ps", bufs=2) as psp,
            tc.tile_pool(name="ev", bufs=3) as evp,
        ):
            ctile = coefp.tile([128, 1], f16)
            nc.vector.memset(ctile[:], COEFF)
            cap = ctile[:]
            wtile = coefp.tile([128, 256], f16)
            nc.gpsimd.dma_start(out=wtile[:], in_=wt[:, :])
            wmain = wtile[:, 0:128]
            w2 = wtile[:, 128:256]

            tile_a = pa.tile([128, L_a], f16)
            tile_b = pb.tile([128, max(b8s, 2)], f16)
            tile_b8 = (pb.tile([128, n8], mybir.dt.float8e4, name="tb8")
                       if b8s < L_pe else None)

            Q = {"sp": nc.sync, "act": nc.scalar, "gp": nc.gpsimd}

            def cb(f):
                return bass.AP(cap.tensor, cap.offset,
                               [[cap.ap[0][0], 128], [0, f]])

            psum_tiles = {}
            ev_tiles = {}
            for name in sorted(order, key=lambda n: started[n]):
                op = ops[name]
                a0, a1 = op["a0"], op["a1"]
                kind = op["kind"]
                if kind == "loadA":
                    Q[op["q"]].dma_start(out=tile_a[:, a0:a1],
                                         in_=xa[:, a0:a1])
                elif kind == "loadB":
                    Q[op["q"]].dma_start(out=tile_b[:, a0:a1],
                                         in_=xb[:, a0:a1])
                elif kind == "loadB8":
                    r0, r1 = a0 - (b8s - 1), a1 - (b8s - 1)
                    Q[op["q"]].dma_start(out=tile_b8[:, r0:r1],
                                         in_=xb8[:, r0:r1])
                elif kind == "scanA":
                    init = 0.0 if op["idx"] == 0 else tile_a[:, a0 - 1:a0]
                    nc.vector.tensor_tensor_scan(
                        out=tile_a[:, a0:a1], data0=cb(a1 - a0),
                        data1=tile_a[:, a0:a1], initial=init,
                        op0=mybir.AluOpType.mult, op1=mybir.AluOpType.add)
                elif kind == "mm":
                    g = op["grp"]
                    if g not in psum_tiles:
                        psum_tiles[g] = psp.tile(
                            [128, cfg["psum_chunk"] * cfg["ev_group"]], f32,
                            name="psg")
                    pt = psum_tiles[g]
                    f = a1 - a0
                    o = (op["idx"] % cfg["ev_group"]) * cfg["psum_chunk"]
                    if a0 >= b8s:
                        r0, r1 = a0 - (b8s - 1), a1 - (b8s - 1)
                        nc.tensor.matmul(pt[:, o:o + f], wmain,
                                         tile_b8[:, r0:r1],
                                         start=True, stop=False)
                        nc.tensor.matmul(pt[:, o:o + f], w2,
                                         tile_b8[:, r0 - 1:r1 - 1],
                                         start=False, stop=True)
                    else:
                        nc.tensor.matmul(pt[:, o:o + f], wmain,
                                         tile_b[:, a0:a1],
                                         start=True, stop=False)
                        nc.tensor.matmul(pt[:, o:o + f], w2,
                                         tile_b[:, a0 - 1:a1 - 1],
                                         start=False, stop=True)
                elif kind == "evac":
                    f = a1 - a0
                    et = evp.tile([128, cfg["psum_chunk"] * cfg["ev_group"]],
                                  f16, name="evt")
                    ev_tiles[op["idx"]] = et
                    pt = psum_tiles.pop(op["idx"])
                    nc.scalar.copy(out=et[:, 0:f], in_=pt[:, 0:f])
                elif kind == "storeA":
                    Q[op["q"]].dma_start(out=ya[:, a0 - W:a1 - W],
                                         in_=tile_a[:, a0:a1])
                else:  # storeB
                    et = ev_tiles[op["idx"]]
                    dst = yb8 if op.get("fp8") else yb
                    Q[op["q"]].dma_start(out=dst[:, a0:a1],
                                         in_=et[:, 0:a1 - a0])


    if split_waits:
        _split_multi_waits(nc, mybir)
    return nc


def _split_multi_waits(nc, mybir):
    """Codegen accepts at most ONE sync wait per instruction; rewrite
    multi-wait instructions into single-wait NoOps on the same queue."""
    for fn in nc.m.functions:
        for blk in fn.blocks:
            out = []
            changed = False
            for inst in blk.instructions:
                si = inst.sync_info
                if si is not None and len(si.on_wait) > 1:
                    waits = list(si.on_wait)
                    for j, w_ in enumerate(waits[:-1]):
                        out.append(
                            mybir.InstNoOp(
                                name=f"splitwait-{inst.name}-{j}",
                                opcode="NoOp",
                                engine=inst.engine,
                                sync_info=mybir.SyncInfo(on_wait=[w_], on_update=[]),
                            )
                        )
                    si.on_wait = [waits[-1]]
                    inst.sync_info = si
                    changed = True
                out.append(inst)
            if changed:
                blk.instructions = out


def _shard_inputs(X, cfg=None):
    cfg = cfg or CFG
    m_a, m_b, c, L_a, L_pe = _derive(cfg)
    Xh = X.astype(np.float16)
    S = NSEG * m_a
    wmain, w2 = _weights()
    wt = np.concatenate([wmain, w2], axis=1)  # [128, 256]
    in_maps = []
    for cr in range(N_CORES):
        rows = Xh[cr * RPC : (cr + 1) * RPC]
        padded = np.concatenate([np.zeros((RPC, W), np.float16), rows], axis=1)
        A = np.empty((RPC, NSEG, L_a), np.float16)
        for i in range(NSEG):
            A[:, i, :] = padded[:, i * m_a : i * m_a + L_a]
        # B: position-major per row: [zero, warm1, warm2, data...]
        B = np.zeros((128, L_pe), np.float16)
        for r in range(RPC):
            rb = r * (c + 3)
            B[:, rb + 1] = rows[r, S - 256 : S - 128]
            B[:, rb + 2] = rows[r, S - 128 : S]
            seg = rows[r, S:]                      # (16*m_b,)
            M = seg.reshape(c, 128).T              # [128, c] position-major
            B[:, rb + 3 : rb + 3 + c] = M
        im = {
            "xa": np.ascontiguousarray(A.reshape(128, L_a)),
            "wt": np.ascontiguousarray(wt),
        }
        b8s = _b8_start(cfg)
        if b8s < L_pe:
            import ml_dtypes
            im["xb"] = np.ascontiguousarray(B[:, :b8s])
            im["xb8"] = np.ascontiguousarray(
                B[:, b8s - 1:].astype(ml_dtypes.float8_e4m3))
        else:
            im["xb"] = B
        in_maps.append(im)
    return in_maps


def _gather_outputs(results, cfg=None):
    cfg = cfg or CFG
    m_a, m_b, c, L_a, L_pe = _derive(cfg)
    out = np.empty((ROWS, N), dtype=np.float32)
    S = NSEG * m_a
    for cr in range(N_CORES):
        OA = results[cr]["ya"].reshape(RPC, NSEG, m_a).astype(np.float32)
        OB = results[cr]["yb"].astype(np.float32)  # [128, L_pe]
        if "yb8" in results[cr]:
            ob8 = results[cr]["yb8"]
            if ob8.dtype == np.uint8:
                import ml_dtypes
                ob8 = ob8.view(ml_dtypes.float8_e4m3)
            ob8 = np.asarray(ob8).astype(np.float32)
            b8start = _fp8_start(cfg)
            if b8start is not None:
                OB[:, b8start:] = ob8[:, b8start:]
        r0 = cr * RPC
        out[r0:r0 + RPC, :S] = OA.reshape(RPC, S)
        for r in range(RPC):
            b0 = r * (c + 3) + 3
            out[r0 + r, S:] = OB[:, b0 : b0 + c].T.reshape(-1)
    return out


def run(X, trace=False):
    from concourse.bass_utils import run_bass_kernel_spmd

    if "nc" not in _cached:
        _cached["nc"] = _build_bass()
    nc = _cached["nc"]
    in_maps = _shard_inputs(np.ascontiguousarray(X, dtype=np.float32))
    try:
        res = run_bass_kernel_spmd(
            nc, in_maps, core_ids=list(range(N_CORES)), trace=trace
        )
    except ModuleNotFoundError:
        import os
        os.environ["BASS_NEVER_TRACE"] = "1"
        res = run_bass_kernel_spmd(
            nc, in_maps, core_ids=list(range(N_CORES)), trace=False
        )
    return _gather_outputs(res.results), res


def kernel(inputs: np.ndarray) -> np.ndarray:
    out, _ = run(inputs, trace=False)
    return out

